# revision 1
# baseline (speedup 1.0000x reference)
"""Trainium2 Bass kernel for multi-head self-attention (B=4, N=2048, C=1024, H=16).

Sharding: 8 cores = 4 batches x 2 head-groups (8 heads each). Each core:
  - computes Q^T/K^T (transposed layout) and V for its 8 heads from x[b]
  - flash-style attention: S^T tiles -> exp -> PV with a fused ones-column
    producing per-query softmax sums in the same matmul
  - normalizes O^T by 1/sum and applies its partial output projection
Host: preps per-core inputs (transpose + bf16 cast + weight column select),
adds the two partial projection outputs per batch (the tensor-parallel
reduce), and concatenates batches. No device collectives.
"""

import numpy as np
import ml_dtypes

import concourse.bass as bass
import concourse.mybir as mybir
import concourse.tile as tile
from concourse import bacc
from concourse.ap import AP
from concourse.bass_utils import run_bass_kernel_spmd

BF16 = mybir.dt.bfloat16
F32 = mybir.dt.float32
F32R = mybir.dt.float32r
Exp = mybir.ActivationFunctionType.Exp
bf = ml_dtypes.bfloat16

B, N, C = 4, 2048, 1024
H, D = 16, 64
N_CORES = 8
HPC = H // 2  # heads per core (8)
PAIRS = HPC // 2  # head pairs per core (4)
CT = C // 128  # contraction tiles over C (8)
KT = N // 128  # key tiles (16)
RT = N // 128  # row tiles for V (16)
QC = N // 1024  # 1024-wide q chunks (2)
QT4 = N // 512  # 512-wide q chunks (4)
SCALE = 1.0 / float(np.sqrt(D))

_COMPILED = {}


def _build(with_bias: bool):
    nc = bacc.Bacc("TRN2", target_bir_lowering=False, debug=False,
                   num_devices=N_CORES)
    xt_d = nc.dram_tensor("xt", [C, N], BF16, kind="ExternalInput").ap()
    wqk_d = nc.dram_tensor("wqk", [C, 1024], BF16, kind="ExternalInput").ap()
    wv_d = nc.dram_tensor("wv", [C, 512], BF16, kind="ExternalInput").ap()
    wpr_d = nc.dram_tensor("wpr", [512, C], BF16, kind="ExternalInput").ap()
    if with_bias:
        bqk_d = nc.dram_tensor("bqk", [1, 1024], BF16, kind="ExternalInput").ap()
        bv_d = nc.dram_tensor("bv", [1, 512], BF16, kind="ExternalInput").ap()
        bpr_d = nc.dram_tensor("bpr", [1, C], BF16, kind="ExternalInput").ap()
    out_d = nc.dram_tensor("out", [N, C], F32, kind="ExternalOutput").ap()

    with tile.TileContext(nc) as tc:
        with (
            tc.tile_pool(name="persist", bufs=1) as pp,
            tc.tile_pool(name="pt", bufs=8) as pt_pool,
            tc.tile_pool(name="stage", bufs=4) as stage_pool,
            tc.tile_pool(name="bc", bufs=4) as bc_pool,
            tc.tile_pool(name="rc", bufs=4) as rc_pool,
            tc.tile_pool(name="oc", bufs=3) as oc_pool,
            tc.tile_pool(name="oa", bufs=3) as oa_pool,
            tc.tile_pool(name="ps_sc", bufs=2, space="PSUM") as ps_sc,
            tc.tile_pool(name="ps_pv", bufs=2, space="PSUM") as ps_pv,
        ):
            # x^T tiles loaded in 512-column chunks so the first V row-tiles
            # can start before the full 4MB lands
            xt_sb = [pp.tile([128, N], BF16, tag=f"xt{ct}", name=f"xt{ct}")
                     for ct in range(CT)]
            wv_sb = [pp.tile([128, 512], BF16, tag=f"wv{ct}", name=f"wv{ct}")
                     for ct in range(CT)]
            for ct in range(CT):
                nc.sync.dma_start(wv_sb[ct][:], wv_d[ct * 128:(ct + 1) * 128, :])
                nc.sync.dma_start(
                    xt_sb[ct][:, 0:512], xt_d[ct * 128:(ct + 1) * 128, 0:512])
            # W_qkv tiles stream in while the V phase runs (full rows keep
            # HBM descriptors at 2KB); interleave with the remaining x^T
            # chunks so the V row-tiles that need x^T chunk q4 aren't queued
            # behind all eight W_qkv loads
            wqk_sb = [pp.tile([128, 1024], BF16, tag=f"wqk{ct}", name=f"wqk{ct}")
                      for ct in range(CT)]
            for q4 in range(1, QT4):
                qsl = slice(q4 * 512, (q4 + 1) * 512)
                for ct in range(CT):
                    nc.sync.dma_start(
                        xt_sb[ct][:, qsl], xt_d[ct * 128:(ct + 1) * 128, qsl])
                for ct in range(q4 - 1, CT, 3):
                    nc.sync.dma_start(wqk_sb[ct][:],
                                      wqk_d[ct * 128:(ct + 1) * 128, :])
            wpr_sb = []
            for cp in range(PAIRS):
                t = pp.tile([128, C], BF16, tag=f"wpr{cp}", name=f"wpr{cp}")
                nc.sync.dma_start(t[:], wpr_d[cp * 128:(cp + 1) * 128, :])
                wpr_sb.append(t)
            if with_bias:
                ones = pp.tile([1, N], BF16, tag="ones")
                nc.vector.memset(ones[:], 1.0)
                bqk_sb = pp.tile([1, 1024], BF16, tag="bqk")
                nc.sync.dma_start(bqk_sb[:], bqk_d[:])
                bv_sb = pp.tile([1, 512], BF16, tag="bv")
                nc.sync.dma_start(bv_sb[:], bv_d[:])
                bpr_sb = pp.tile([1, C], BF16, tag="bpr")
                nc.sync.dma_start(bpr_sb[:], bpr_d[:])

            ones16 = pp.tile([1, 64], BF16, tag="ones16")
            nc.vector.memset(ones16[:], 1.0)

            qt_sb = [pp.tile([128, N], BF16, tag=f"qt{p}", name=f"qt{p}")
                     for p in range(PAIRS)]
            kt_sb = [pp.tile([128, N], BF16, tag=f"kt{p}", name=f"kt{p}")
                     for p in range(PAIRS)]
            va_sb = [pp.tile([128, HPC * 65], BF16, tag=f"va{rt}", name=f"va{rt}")
                     for rt in range(RT)]
            plhsT = [pp.tile([128, N], BF16, tag=f"pl{p}", name=f"pl{p}")
                     for p in range(PAIRS)]

            # ---- V rows first: out[r, hd] = sum_c xT[c, r] * Wv[c, hd],
            # interleaved with a ones column per head (col h*65+64) that makes
            # the PV matmul also produce the softmax row-sums.
            for rt in range(RT):
                nc.vector.memset(va_sb[rt][:], 1.0)
                rsl = slice(rt * 128, (rt + 1) * 128)
                acc = ps_sc.tile([128, 1024], F32, tag="sc", name="acc_v")
                for ct in range(CT):
                    nc.tensor.matmul(acc[:, 0:512], xt_sb[ct][:, rsl],
                                     wv_sb[ct][:],
                                     start=(ct == 0), stop=(not with_bias and ct == CT - 1))
                if with_bias:
                    nc.tensor.matmul(acc[:, 0:512], ones[0:1, 0:128], bv_sb[:],
                                     start=False, stop=True)
                dst3 = va_sb[rt].rearrange("p (h d) -> p h d", d=65)[:, :, 0:64]
                src3 = acc[:, 0:512].rearrange("p (h d) -> p h d", d=64)
                nc.vector.tensor_copy(dst3, src3)

            # ---- Q^T / K^T packs for one pair: out[o, q] = sum_c W[c,o]*xT[c,q]
            def qk_chunk(p, i):
                dst, col0 = ((qt_sb, 0), (kt_sb, 512))[i // QT4]
                q4 = i % QT4
                osl = slice(col0 + p * 128, col0 + (p + 1) * 128)
                qsl = slice(q4 * 512, (q4 + 1) * 512)
                acc = ps_sc.tile([128, 1024], F32, tag="sc", name="acc_qk")
                for ct in range(CT):
                    nc.tensor.matmul(acc[:, 0:512], wqk_sb[ct][:, osl],
                                     xt_sb[ct][:, qsl], start=(ct == 0),
                                     stop=(not with_bias and ct == CT - 1))
                if with_bias:
                    nc.tensor.matmul(acc[:, 0:512], bqk_sb[0:1, osl],
                                     ones[0:1, qsl], start=False, stop=True)
                nc.vector.tensor_copy(dst[p][:, qsl], acc[:, 0:512])

            def qk_pack(p):
                for i in range(2 * QT4):
                    qk_chunk(p, i)

            # ---- attention for (pair, 1024-wide q chunk) ----
            def attention(p, qc, filler=None):
                qsl = slice(qc * 1024, (qc + 1) * 1024)
                o_ps = [ps_pv.tile([65, 1024], F32, tag="pv",
                                   name=f"o_ps{p}_{qc}_{i}") for i in range(2)]
                for kt in range(KT):
                    if filler is not None:
                        filler(kt)
                    ksl = slice(kt * 128, (kt + 1) * 128)
                    # interleave the two heads' score matmuls so the
                    # row-group-0 and row-group-64 instructions are adjacent
                    # and run concurrently in the PE array
                    scs = []
                    for hl in range(2):
                        scs.append(ps_sc.tile([128, 1024], F32, tag="sc",
                                              name="sc_att"))
                    for qh in range(2):
                        q0 = qc * 1024 + qh * 512
                        for hl in range(2):
                            pb = hl * 64
                            nc.tensor.matmul(
                                scs[hl][:, qh * 512:(qh + 1) * 512],
                                kt_sb[p][pb:pb + 64, ksl],
                                qt_sb[p][pb:pb + 64, q0:q0 + 512],
                                start=True, stop=True)
                    pts = []
                    for hl in range(2):
                        pt = pt_pool.tile([128, 1024], BF16, tag="pt", name="pt")
                        nc.scalar.activation(pt[:], scs[hl][:], Exp, scale=SCALE)
                        pts.append(pt)
                    for hl in range(2):
                        lh = 2 * p + hl
                        for qh in range(2):
                            nc.tensor.matmul(
                                o_ps[hl][:, qh * 512:(qh + 1) * 512],
                                va_sb[kt][:, lh * 65:(lh + 1) * 65],
                                pts[hl][:, qh * 512:(qh + 1) * 512],
                                start=(kt == 0), stop=(kt == KT - 1))
                # normalize O^T by 1/rowsum, store as proj lhsT (bf16).
                # First a quick PSUM->SBUF copy so the PV PSUM slot frees
                # immediately and the next segment's matmuls can start.
                # The chain is split into 512-wide halves so the projection
                # tiles gated on it start one half-chain earlier.
                last_seg = (p == PAIRS - 1 and qc == QC - 1)
                for hl in range(2):
                    oa = oa_pool.tile([65, 1024], F32, tag="oa", name="oa")
                    if last_seg:
                        # ACT is idle once the final exps drain; doing the
                        # copy there unblocks the DVE recip chain that gates
                        # the tail projection tiles
                        nc.scalar.copy(oa[:], o_ps[hl][:])
                    else:
                        nc.vector.tensor_copy(oa[:], o_ps[hl][:])
                    for qh in range(2):
                        hs = slice(qh * 512, (qh + 1) * 512)
                        gs = slice(qc * 1024 + qh * 512,
                                   qc * 1024 + (qh + 1) * 512)
                        recip = rc_pool.tile([1, 512], F32, tag="rc",
                                             name="recip")
                        nc.vector.reciprocal(recip[:], oa[64:65, hs])
                        bcst = bc_pool.tile([64, 512], F32, tag="bc",
                                            name="bcst")
                        if last_seg:
                            # attention is over, so the scores PSUM slots are
                            # free: broadcast on the idle PE via a ones-matmul
                            # instead of the port-bound DMA replicate
                            recip16 = rc_pool.tile([1, 512], BF16, tag="rc16",
                                                   name="recip16")
                            nc.vector.tensor_copy(recip16[:], recip[:])
                            bc_ps = ps_sc.tile([128, 1024], F32, tag="sc",
                                               name="bc_ps")
                            nc.tensor.matmul(bc_ps[0:64, 0:512], ones16[0:1, :],
                                             recip16[0:1, :],
                                             start=True, stop=True)
                            nc.vector.tensor_copy(bcst[:], bc_ps[0:64, 0:512])
                        else:
                            # broadcast 1/sum to 64 partitions (0-step DMA)
                            src = AP(recip.tensor, recip.offset,
                                     [recip.ap[0], [0, 64]] + list(recip.ap[1:]))
                            nc.sync.dma_start(bcst[:], src)
                        if hl == 0:
                            nc.vector.tensor_mul(plhsT[p][0:64, gs],
                                                 oa[0:64, hs], bcst[:])
                        else:
                            st = stage_pool.tile([64, 512], BF16, tag="st",
                                                 name="st")
                            nc.vector.tensor_mul(st[:], oa[0:64, hs], bcst[:])
                            if last_seg:
                                # tail-critical: spread the partition move
                                # over two DMA queues
                                for pi in range(2):
                                    ps_ = slice(pi * 32, (pi + 1) * 32)
                                    nc.sync.dma_start(
                                        plhsT[p][64 + pi * 32:96 + pi * 32, gs],
                                        st[ps_, :])
                            else:
                                nc.sync.dma_start(plhsT[p][64:128, gs], st[:])

            # ---- partial output projection for one 128-row q tile ----
            def proj_tile(qt_i):
                qsl = slice(qt_i * 128, (qt_i + 1) * 128)
                oc = oc_pool.tile([128, 1024], F32, tag="oc", name="oc")
                for nch in range(2):
                    nsl = slice(nch * 512, (nch + 1) * 512)
                    acc = ps_sc.tile([128, 1024], F32, tag="sc", name="acc_pr")
                    for cp in range(PAIRS):
                        nc.tensor.matmul(acc[:, 0:512], plhsT[cp][:, qsl],
                                         wpr_sb[cp][:, nsl], start=(cp == 0),
                                         stop=(not with_bias and cp == PAIRS - 1))
                    if with_bias:
                        nc.tensor.matmul(acc[:, 0:512], ones[0:1, 0:128],
                                         bpr_sb[0:1, nsl], start=False,
                                         stop=True)
                    nc.vector.tensor_copy(oc[:, nsl], acc[:, 0:512])
                # 512KB output row-block split over DMA queues; the final
                # tiles sit on the kernel tail, so spread them 4-wide
                nsplit = 4 if qt_i >= 14 else 2
                for oi in range(nsplit):
                    osl = slice(oi * (1024 // nsplit), (oi + 1) * (1024 // nsplit))
                    nc.sync.dma_start(out_d[qsl, osl], oc[:, osl])

            def proj_filler(p, kt):
                # qc0's projection tiles fill the ACT-bound gaps of qc1
                if kt in (5, 11):
                    proj_tile(2 * p + (kt > 5))

            qk_pack(0)
            for p in range(PAIRS):
                if p + 1 < PAIRS:
                    qk_pack(p + 1)  # overlaps with attention of pair p
                attention(p, 0)
            for p in range(PAIRS):
                attention(p, 1, filler=lambda kt, p=p: proj_filler(p, kt))
            for qt_i in range(8, 16):
                proj_tile(qt_i)

    nc.compile()
    return nc


def _get_nc(with_bias=False):
    if with_bias not in _COMPILED:
        _COMPILED[with_bias] = _build(with_bias)
    return _COMPILED[with_bias]


def _prep_in_maps(x, W_qkv, b_qkv, W_proj, b_proj, with_bias):
    in_maps = []
    for c in range(N_CORES):
        b = c // 2
        g = c % 2
        hs = slice(g * 512, (g + 1) * 512)
        xt = np.ascontiguousarray(x[b].T).astype(bf)
        wq = W_qkv[:, 0:C][:, hs]
        wk = W_qkv[:, C:2 * C][:, hs]
        wv = W_qkv[:, 2 * C:3 * C][:, hs]
        wqk = np.ascontiguousarray(np.concatenate([wq, wk], axis=1)).astype(bf)
        wpr = np.ascontiguousarray(W_proj[hs, :]).astype(bf)
        m = {
            "xt": xt, "wqk": wqk, "wv": np.ascontiguousarray(wv).astype(bf),
            "wpr": wpr,
        }
        if with_bias:
            bq = b_qkv[0:C][hs]
            bk = b_qkv[C:2 * C][hs]
            bvv = b_qkv[2 * C:3 * C][hs]
            m["bqk"] = np.concatenate([bq, bk])[None, :].astype(bf)
            m["bv"] = np.ascontiguousarray(bvv[None, :]).astype(bf)
            m["bpr"] = ((b_proj if g == 0 else np.zeros_like(b_proj))
                        [None, :].astype(bf))
        in_maps.append(m)
    return in_maps


def kernel(x, W_qkv, b_qkv, W_proj, b_proj):
    x = np.asarray(x, dtype=np.float32)
    W_qkv = np.asarray(W_qkv, dtype=np.float32)
    b_qkv = np.asarray(b_qkv, dtype=np.float32)
    W_proj = np.asarray(W_proj, dtype=np.float32)
    b_proj = np.asarray(b_proj, dtype=np.float32)
    with_bias = bool(np.any(b_qkv) or np.any(b_proj))
    nc = _get_nc(with_bias)
    in_maps = _prep_in_maps(x, W_qkv, b_qkv, W_proj, b_proj, with_bias)
    res = run_bass_kernel_spmd(nc, in_maps, core_ids=list(range(N_CORES)))
    out = np.empty((B, N, C), dtype=np.float32)
    for b in range(B):
        out[b] = res.results[2 * b]["out"] + res.results[2 * b + 1]["out"]
    return out



# revision 17
# speedup vs baseline: 1.2552x; 1.2552x over previous
"""Trainium2 Bass kernel for multi-head self-attention (B=4, N=2048, C=1024, H=16).

Sharding: 8 cores = 4 batches x 2 head-groups (8 heads each). Each core:
  - computes Q^T/K^T (transposed layout) and V for its 8 heads from x[b]
  - flash-style attention with the exp'd score tile P[k,q] as the matmul
    STATIONARY and V[k,d] as the moving operand, so each PV accumulation
    step costs 65 moving columns instead of 512 (the cost model charges
    only the moving/free dimension). A fused ones-column in V produces
    per-query softmax sums in the same matmul.
  - normalizes O[q,d] by 1/sum (per-partition scalar multiply), transposes
    each [128,128] block via the DMA XBAR into the projection lhsT layout,
    and applies its partial output projection.
The whole kernel is one software-pipelined stream over (segment, kt) slots:
the scores+exp stream runs two slots ahead of the PV stream, crossing
segment boundaries, and ~2-matmul filler fragments (QKV prep / projection)
are woven into each slot so neither the PE nor the activation engine waits.
Inputs land in merged SBUF tiles so each tensor is one or two wide DMAs.
Host: preps per-core inputs (transpose + bf16 cast + weight column select),
adds the two partial projection outputs per batch (the tensor-parallel
reduce), and concatenates batches. No device collectives.
"""

import os
import numpy as np
import ml_dtypes

import concourse.bass as bass
import concourse.mybir as mybir
import concourse.tile as tile
from concourse import bacc
from concourse.ap import AP
from concourse.bass_utils import run_bass_kernel_spmd

BF16 = mybir.dt.bfloat16
F32 = mybir.dt.float32
Exp = mybir.ActivationFunctionType.Exp
bf = ml_dtypes.bfloat16

B, N, C = 4, 2048, 1024
H, D = 16, 64
N_CORES = 8
HPC = H // 2  # heads per core (8)
PAIRS = HPC // 2  # head pairs per core (4)
CT = C // 128  # contraction tiles over C (8)
KT = N // 128  # key tiles (16)
RT = N // 128  # row tiles for V (16)
QCH = N // 512  # 512-wide q chunks per pair (4)
SCALE = 1.0 / float(np.sqrt(D))

_COMPILED = {}
KFLAGS = set(os.environ.get("KFLAGS", "").split(","))


def _build(with_bias: bool):
    nc = bacc.Bacc("TRN2", target_bir_lowering=False, debug=False,
                   num_devices=N_CORES)
    xt_d = nc.dram_tensor("xt", [C, N], BF16, kind="ExternalInput").ap()
    wqk_d = nc.dram_tensor("wqk", [C, 1024], BF16, kind="ExternalInput").ap()
    wv_d = nc.dram_tensor("wv", [C, 512], BF16, kind="ExternalInput").ap()
    wpr_d = nc.dram_tensor("wpr", [512, C], BF16, kind="ExternalInput").ap()
    if with_bias:
        bqk_d = nc.dram_tensor("bqk", [1, 1024], BF16, kind="ExternalInput").ap()
        bv_d = nc.dram_tensor("bv", [1, 512], BF16, kind="ExternalInput").ap()
        bpr_d = nc.dram_tensor("bpr", [1, C], BF16, kind="ExternalInput").ap()
    out_d = nc.dram_tensor("out", [N, C], F32, kind="ExternalOutput").ap()

    with tile.TileContext(nc) as tc:
        with (
            tc.tile_pool(name="persist", bufs=1) as pp,
            tc.tile_pool(name="pt", bufs=6) as pt_pool,
            tc.tile_pool(name="nm", bufs=6) as nm_pool,
            tc.tile_pool(name="rc", bufs=4) as rc_pool,
            tc.tile_pool(name="oc", bufs=4) as oc_pool,
            tc.tile_pool(name="ps_sc", bufs=2, space="PSUM") as ps_sc,
            tc.tile_pool(name="ps_pv", bufs=2, space="PSUM") as ps_pv,
        ):
            # ---- PE p-state warm-up: the tensor engine needs ~3us of
            # continuous execution to reach full clock; run dummy matmuls
            # while the input DMAs are in flight ----
            if "nowarm" not in KFLAGS:
                warm = pp.tile([1, 512], BF16, tag="warm")
                nc.vector.memset(warm[:], 0.0)
                wacc = ps_sc.tile([128, 1024], F32, tag="sc", name="wacc")
                for _ in range(14):
                    nc.tensor.matmul(wacc[0:1, 0:512], warm[0:1, 0:1], warm[:],
                                     start=True, stop=True)

            # ---- persistent SBUF tiles, merged per tensor so each input is
            # one or two wide DMAs (a DMA moves all 128 partitions in
            # parallel; instruction count is what costs HWDGE time) ----
            xt_sb = pp.tile([128, CT, N], BF16, tag="xt")
            wv_sb = pp.tile([128, CT, 512], BF16, tag="wv")
            wqk_sb = pp.tile([128, CT, 1024], BF16, tag="wqk")
            wpr_sb = pp.tile([128, PAIRS, C], BF16, tag="wpr")
            xt_r = xt_d.rearrange("(ct p) n -> p ct n", p=128)
            wqk_r = wqk_d.rearrange("(ct p) o -> p ct o", p=128)
            wv_r = wv_d.rearrange("(ct p) o -> p ct o", p=128)
            wpr_r = wpr_d.rearrange("(cp p) o -> p cp o", p=128)

            # K columns first (the first QK chunk the schedule emits is a
            # K^T chunk), then Q; xt chunk0 + wv on the ACT queue in parallel
            nc.sync.dma_start(wqk_sb[:, :, 512:1024], wqk_r[:, :, 512:1024])
            nc.sync.dma_start(wqk_sb[:, :, 0:512], wqk_r[:, :, 0:512])
            dmaq2 = nc.sync if "noactdma" in KFLAGS else nc.scalar
            dmaq2.dma_start(xt_sb[:, :, 0:512], xt_r[:, :, 0:512])
            dmaq2.dma_start(wv_sb[:], wv_r[:])
            for q4 in range(1, 4):
                qsl = slice(q4 * 512, (q4 + 1) * 512)
                nc.sync.dma_start(xt_sb[:, :, qsl], xt_r[:, :, qsl])
            nc.sync.dma_start(wpr_sb[:], wpr_r[:])
            if with_bias:
                ones = pp.tile([1, N], BF16, tag="ones")
                nc.vector.memset(ones[:], 1.0)
                bqk_sb = pp.tile([1, 1024], BF16, tag="bqk")
                nc.sync.dma_start(bqk_sb[:], bqk_d[:])
                bv_sb = pp.tile([1, 512], BF16, tag="bv")
                nc.sync.dma_start(bv_sb[:], bv_d[:])
                bpr_sb = pp.tile([1, C], BF16, tag="bpr")
                nc.sync.dma_start(bpr_sb[:], bpr_d[:])

            qt_sb = [pp.tile([128, N], BF16, tag=f"qt{p}", name=f"qt{p}")
                     for p in range(PAIRS)]
            kt_sb = [pp.tile([128, N], BF16, tag=f"kt{p}", name=f"kt{p}")
                     for p in range(PAIRS)]
            va_sb = [pp.tile([128, HPC * 65], BF16, tag=f"va{rt}", name=f"va{rt}")
                     for rt in range(RT)]
            plhsT = [pp.tile([128, N], BF16, tag=f"pl{p}", name=f"pl{p}")
                     for p in range(PAIRS)]

            # ---- filler generators: each yield point is ~2 matmuls of PE
            # work, woven into the attention slot stream ----

            def v_tile(rt):
                nc.vector.memset(va_sb[rt][:], 1.0)
                rsl = slice(rt * 128, (rt + 1) * 128)
                acc = ps_sc.tile([128, 512], F32, tag="acc", name="acc_v")
                for ct in range(CT):
                    nc.tensor.matmul(acc[:], xt_sb[:, ct, rsl],
                                     wv_sb[:, ct, :],
                                     start=(ct == 0),
                                     stop=(not with_bias and ct == CT - 1))
                    if ct % 2 == 1 and ct < CT - 1:
                        yield
                if with_bias:
                    nc.tensor.matmul(acc[:], ones[0:1, 0:128], bv_sb[:],
                                     start=False, stop=True)
                dst3 = va_sb[rt].rearrange("p (h d) -> p h d", d=65)[:, :, 0:64]
                src3 = acc[:].rearrange("p (h d) -> p h d", d=64)
                nc.vector.tensor_copy(dst3, src3)

            def qk_chunk(p, i):
                dst, col0 = ((qt_sb, 0), (kt_sb, 512))[i // QCH]
                q4 = i % QCH
                osl = slice(col0 + p * 128, col0 + (p + 1) * 128)
                qsl = slice(q4 * 512, (q4 + 1) * 512)
                acc = ps_sc.tile([128, 512], F32, tag="acc", name="acc_qk")
                for ct in range(CT):
                    nc.tensor.matmul(acc[:], wqk_sb[:, ct, osl],
                                     xt_sb[:, ct, qsl], start=(ct == 0),
                                     stop=(not with_bias and ct == CT - 1))
                    if ct % 2 == 1 and ct < CT - 1:
                        yield
                if with_bias:
                    nc.tensor.matmul(acc[:], bqk_sb[0:1, osl],
                                     ones[0:1, qsl], start=False, stop=True)
                nc.vector.tensor_copy(dst[p][:, qsl], acc[:])

            def proj_half(qt_i, nch, act_copy=False):
                qsl = slice(qt_i * 128, (qt_i + 1) * 128)
                nsl = slice(nch * 512, (nch + 1) * 512)
                oc = oc_pool.tile([128, 512], F32, tag="oc", name="oc")
                acc = ps_sc.tile([128, 512], F32, tag="acc", name="acc_pr")
                for cp in range(PAIRS):
                    nc.tensor.matmul(acc[:], plhsT[cp][:, qsl],
                                     wpr_sb[:, cp, nsl], start=(cp == 0),
                                     stop=(not with_bias and cp == PAIRS - 1))
                    if cp == 1:
                        yield
                if with_bias:
                    nc.tensor.matmul(acc[:], ones[0:1, 0:128],
                                     bpr_sb[0:1, nsl], start=False,
                                     stop=True)
                if act_copy:
                    nc.scalar.copy(oc[:], acc[:])
                else:
                    nc.vector.tensor_copy(oc[:], acc[:])
                nc.sync.dma_start(out_d[qsl, nsl], oc[:])

            def proj_tile(qt_i, act_copy=False):
                yield from proj_half(qt_i, 0, act_copy)
                yield from proj_half(qt_i, 1, act_copy)

            class Fillers:
                def __init__(self):
                    self.gens = []

                def add(self, *gens):
                    self.gens.extend(gens)

                def step(self, n=1):
                    for _ in range(n):
                        while self.gens:
                            try:
                                next(self.gens[0])
                                break
                            except StopIteration:
                                self.gens.pop(0)

            fillers = Fillers()

            def run_gen(g):
                for _ in g:
                    pass

            # ---- segment table: (pair, q_lo, q_width); the last 512-wide
            # chunk of pair 3 is split into two 256-wide minis so the final
            # projections start sooner ----
            segs = [(0, 0, 512), (0, 512, 512), (1, 0, 512), (1, 512, 512),
                    (2, 0, 512), (2, 512, 512), (3, 0, 512), (3, 512, 512),
                    (0, 1024, 512), (1, 1024, 512), (2, 1024, 512),
                    (3, 1024, 512), (0, 1536, 512), (1, 1536, 512),
                    (2, 1536, 512), (3, 1536, 512)]
            NSEG = len(segs)
            if os.environ.get("NSEGR"):
                NSEG = min(NSEG, int(os.environ["NSEGR"]))
                segs = segs[:NSEG]
            pv_of = {}
            pts = {}

            def scores_exp(si, kt):
                p, q_lo, w = segs[si]
                ksl = slice(kt * 128, (kt + 1) * 128)
                sc = ps_sc.tile([128, 1024], F32, tag="sc", name="sc_att")
                for hl in range(2):
                    pb = hl * 64
                    nc.tensor.matmul(
                        sc[:, hl * w:(hl + 1) * w],
                        kt_sb[p][pb:pb + 64, ksl],
                        qt_sb[p][pb:pb + 64, q_lo:q_lo + w],
                        start=(hl == 0 or w == 512),
                        stop=(hl == 1 or w == 512))
                pt = pt_pool.tile([128, 1024], BF16, tag="pt", name="pt")
                nc.scalar.activation(pt[:, 0:2 * w], sc[:, 0:2 * w], Exp,
                                     scale=SCALE)
                pts[(si, kt)] = pt

            def pv_step(si, kt):
                p, q_lo, w = segs[si]
                nqs = w // 128
                if kt == 0:
                    pv_of[si] = [ps_pv.tile([128, 260], F32, tag="pv",
                                            name=f"pv{hl}") for hl in range(2)]
                pv = pv_of[si]
                pt = pts.pop((si, kt))
                for hl in range(2):
                    lh = 2 * p + hl
                    for qs in range(nqs):
                        nc.tensor.matmul(
                            pv[hl][:, qs * 65:(qs + 1) * 65],
                            pt[:, hl * w + qs * 128:hl * w + (qs + 1) * 128],
                            va_sb[kt][:, lh * 65:(lh + 1) * 65],
                            start=(kt == 0 and qs == 0),
                            stop=(kt == KT - 1 and qs == nqs - 1))

            def finish_segment(si, qs_major=False):
                # normalize O by 1/rowsum into [128 q, 128 hd] bf16 tiles,
                # then XBAR-transpose each into plhsT (no PE/PSUM involved).
                # hl-major frees the pv psum slots sooner (next segment's PV
                # waits on them); qs-major unblocks the tail projections.
                p, q_lo, w = segs[si]
                nqs = w // 128
                pv = pv_of.pop(si)
                nms = [nm_pool.tile([128, 128], BF16, tag="nm", name="nm")
                       for _ in range(nqs)]

                def hl_recip(hl):
                    sums = pv[hl].rearrange(
                        "p (s c) -> p s c", c=65)[:, 0:nqs, 64:65]
                    recip = rc_pool.tile([128, 4], F32, tag="rc", name="recip")
                    nc.vector.reciprocal(recip[:, 0:nqs], sums)
                    return recip

                if qs_major:
                    recips = [hl_recip(0), hl_recip(1)]
                    for qs in range(nqs):
                        for hl in range(2):
                            nc.vector.tensor_scalar_mul(
                                nms[qs][:, hl * 64:(hl + 1) * 64],
                                pv[hl][:, qs * 65:qs * 65 + 64],
                                recips[hl][:, qs:qs + 1])
                        qt_i = q_lo // 128 + qs
                        if "noxbar" in KFLAGS:
                            nc.sync.dma_start(
                                plhsT[p][:, qt_i * 128:(qt_i + 1) * 128],
                                nms[qs][:])
                        else:
                            nc.sync.dma_start_transpose(
                                plhsT[p][:, qt_i * 128:(qt_i + 1) * 128],
                                nms[qs][:])
                else:
                    for hl in range(2):
                        recip = hl_recip(hl)
                        for qs in range(nqs):
                            nc.vector.tensor_scalar_mul(
                                nms[qs][:, hl * 64:(hl + 1) * 64],
                                pv[hl][:, qs * 65:qs * 65 + 64],
                                recip[:, qs:qs + 1])
                    for qs in range(nqs):
                        qt_i = q_lo // 128 + qs
                        if "noxbar" in KFLAGS:
                            nc.sync.dma_start(
                                plhsT[p][:, qt_i * 128:(qt_i + 1) * 128],
                                nms[qs][:])
                        else:
                            nc.sync.dma_start_transpose(
                                plhsT[p][:, qt_i * 128:(qt_i + 1) * 128],
                                nms[qs][:])

            # ---- filler supply and per-slot pop counts ----
            V = v_tile
            Q = qk_chunk
            adds = [
                [V(0), V(1), V(2), V(3), Q(0, 5), V(4), V(5), Q(0, 6), V(6),
                 V(7), V(8), Q(0, 7), V(9), V(10), V(11), Q(0, 1), V(12),
                 V(13), V(14), V(15)],
                [Q(1, 4), Q(1, 5), Q(1, 6), Q(1, 7), Q(1, 0), Q(1, 1)],
                [Q(2, 4), Q(2, 5), Q(2, 6), Q(2, 7), Q(2, 0), Q(2, 1)],
                [Q(3, 4), Q(3, 5), Q(3, 6), Q(3, 7), Q(3, 0), Q(3, 1)],
                [Q(0, 2), Q(1, 2), Q(2, 2)],
                [Q(3, 2), Q(0, 3)],
                [Q(1, 3), Q(2, 3)],
                [Q(3, 3), proj_tile(0)],
                [proj_tile(1), proj_tile(2)],
                [proj_tile(3), proj_tile(4)],
                [proj_tile(5), proj_tile(6)],
                [proj_tile(7)],
                [proj_tile(8), proj_tile(9)],
                [proj_tile(10)],
                [proj_tile(11)],
                [],
            ]
            pops = [
                [7, 7, 7, 7, 5, 5, 5, 5, 5, 5, 5, 4, 4, 4, 4, 4],
                [2, 2, 2, 2, 2, 2, 2, 2, 1, 1, 1, 1, 1, 1, 1, 1],
                [2, 2, 2, 2, 2, 2, 2, 2, 1, 1, 1, 1, 1, 1, 1, 1],
                [2, 2, 2, 2, 2, 2, 2, 2, 1, 1, 1, 1, 1, 1, 1, 1],
                [1] * 16,
                [1] * 16,
                [1] * 16,
                [1] * 16,
                [1] * 16,
                [1] * 16,
                [1] * 16,
                [1] * 16,
                [1] * 16,
                [1] * 16,
                [1] * 16,
                [1] * 16,
            ]

            # ---- the pipelined stream: scores+exp two slots ahead of PV ----
            run_gen(qk_chunk(0, 4))
            run_gen(qk_chunk(0, 0))
            total = NSEG * KT
            for s in range(total + 2):
                if s < total:
                    si, kt = divmod(s, KT)
                    if kt == 0:
                        fillers.add(*adds[si])
                    scores_exp(si, kt)
                if s >= 2:
                    sj, kj = divmod(s - 2, KT)
                    pv_step(sj, kj)
                    if kj == KT - 1:
                        finish_segment(sj, qs_major=(sj == NSEG - 1))
                if s < total:
                    fillers.step(pops[si][kt])
            if NSEG == 16:
                for qt_i in range(12, 16):
                    run_gen(proj_tile(qt_i, act_copy=(qt_i >= 14)))

    nc.compile()
    return nc


def _get_nc(with_bias=False):
    if with_bias not in _COMPILED:
        _COMPILED[with_bias] = _build(with_bias)
    return _COMPILED[with_bias]


def _prep_in_maps(x, W_qkv, b_qkv, W_proj, b_proj, with_bias):
    in_maps = []
    for c in range(N_CORES):
        b = c // 2
        g = c % 2
        hs = slice(g * 512, (g + 1) * 512)
        xt = np.ascontiguousarray(x[b].T).astype(bf)
        wq = W_qkv[:, 0:C][:, hs]
        wk = W_qkv[:, C:2 * C][:, hs]
        wv = W_qkv[:, 2 * C:3 * C][:, hs]
        wqk = np.ascontiguousarray(np.concatenate([wq, wk], axis=1)).astype(bf)
        wpr = np.ascontiguousarray(W_proj[hs, :]).astype(bf)
        m = {
            "xt": xt, "wqk": wqk, "wv": np.ascontiguousarray(wv).astype(bf),
            "wpr": wpr,
        }
        if with_bias:
            bq = b_qkv[0:C][hs]
            bk = b_qkv[C:2 * C][hs]
            bvv = b_qkv[2 * C:3 * C][hs]
            m["bqk"] = np.concatenate([bq, bk])[None, :].astype(bf)
            m["bv"] = np.ascontiguousarray(bvv[None, :]).astype(bf)
            m["bpr"] = ((b_proj if g == 0 else np.zeros_like(b_proj))
                        [None, :].astype(bf))
        in_maps.append(m)
    return in_maps


def kernel(x, W_qkv, b_qkv, W_proj, b_proj):
    x = np.asarray(x, dtype=np.float32)
    W_qkv = np.asarray(W_qkv, dtype=np.float32)
    b_qkv = np.asarray(b_qkv, dtype=np.float32)
    W_proj = np.asarray(W_proj, dtype=np.float32)
    b_proj = np.asarray(b_proj, dtype=np.float32)
    with_bias = bool(np.any(b_qkv) or np.any(b_proj))
    nc = _get_nc(with_bias)
    in_maps = _prep_in_maps(x, W_qkv, b_qkv, W_proj, b_proj, with_bias)
    res = run_bass_kernel_spmd(nc, in_maps, core_ids=list(range(N_CORES)))
    out = np.empty((B, N, C), dtype=np.float32)
    for b in range(B):
        out[b] = res.results[2 * b]["out"] + res.results[2 * b + 1]["out"]
    return out
